# revision 57
# baseline (speedup 1.0000x reference)
"""YOLO-style loss kernel for Trainium2, data-parallel over 8 NeuronCores.

v7: HW-proven op set only. fp8(e4m3) payload for cls AND wh rows (the
dtype-blind Pool engine is their only reader; halves their DMA bytes),
two DMA queues (SP + ACT) with box1 rows on ACT so both IoU chains
start by ~6us, ACT dedicated to Square+accumulate reductions, per-box
pipelined IoU chain, |x| via min/negate(t_s 4x)/max, late squares
split across Pool-mult + DVE tensor_scalar-accumulate.

Host prep is layout-only (transpose, fp16/fp8 cast, constant scale of
xy rows by k=2/S); all data-dependent math is on device.

Per core: xin[128, 12, 784] fp16 rows:
  0:4   Tc,Pc [tc0, tc1, pc0, pc1]
  4:8   xy box0 [k*px0, k*py0, k*tx0, k*ty0]
  8:12  xy box1
x8[128, 48, 784] fp8e4m3 rows:
  0:4   wh box0 [pw0, ph0, tw0, th0]
  4:8   wh box1
  8:48  cls tiles, per tile (nd ch): nd p-rows + nd t-rows

Math (scale-free responsible-box form):
  iw2 = (pw+tw) - max(|k dxy|, |dwh|); i_j = relu(iw2_w)*relu(iw2_h)
  s_j = pw*ph + tw*th;  resp0 = [i0*s1 >= i1*s0]
  m_j = obj*resp_j; mo = m swapped
  noobj conf = sum of (noobj-masked dconf)^2 directly (is_le mask)

acc[P,16] fp32 (host weights + partition sum on host):
  0 xyD (LC/k^2)  10 xyP (LC/k^2)  1 c (1)  2 noobj dconf^2 (+0.5)
  4..7 cls t0-t3 (1)  8 cls t4+t5 merged (1)
"""

import numpy as np

N_CORES = 8
BATCH = 16384
S = 7
P = 128
SHARD = BATCH // N_CORES
CELLS = SHARD * S * S
CPP = CELLS // P                  # 784
K_XY = 2.0 / S

LAMBDA_COORD = 5.0
LAMBDA_NOOBJ = 0.5

_CACHE = {}

# cls tiles: (n_channels, mask_eng); sub: pool, sq: act
CLS_PLAN = [
    (4, "pool"),
    (4, "pool"),
    (4, "dve"),
    (4, "dve"),
    (2, "dve"),
    (2, "dve"),
]
assert sum(t[0] for t in CLS_PLAN) == 20


def _split_waits(nc, max_waits=1):
    import concourse.mybir as mybir

    n_new = 0
    for fn in nc.m.functions:
        for blk in fn.blocks:
            out = []
            changed = False
            for inst in list(blk.instructions):
                si = inst.sync_info
                ow = list(si.on_wait) if si is not None else []
                if len(ow) > max_waits:
                    for w in ow[:-max_waits]:
                        n_new += 1
                        out.append(
                            mybir.InstNoOp(
                                name=f"waitnop-{n_new}-{inst.name}",
                                engine=inst.engine,
                                ins=[],
                                outs=[],
                                sync_info=mybir.SyncInfo(
                                    on_wait=[w], on_update=[]
                                ),
                            )
                        )
                    inst.sync_info = mybir.SyncInfo(
                        on_wait=ow[-max_waits:], on_update=list(si.on_update)
                    )
                    changed = True
                out.append(inst)
            if changed:
                blk.instructions = out
    return n_new


def _build_bass():
    import concourse.bass as bass
    import concourse.mybir as mybir
    from contextlib import ExitStack
    from concourse.tile import TileContext

    f8 = mybir.dt.float8e4
    f16 = mybir.dt.float16
    f32 = mybir.dt.float32
    Op = mybir.AluOpType
    Act = mybir.ActivationFunctionType
    C = CPP
    NT = len(CLS_PLAN)

    nc = bass.Bass()
    # xin fp16: 0:4 conf [tc0,tc1,pc0,pc1], 4:8 xy box0, 8:12 xy box1
    # x8 fp8: 0:4 wh box0, 4:8 wh box1, 8:48 cls tiles
    # (wh rides fp8 because only the dtype-blind Pool engine reads it)
    xin = nc.dram_tensor("xin", [P, 12, C], f16, kind="ExternalInput")
    x8 = nc.dram_tensor("x8", [P, 48, C], f8, kind="ExternalInput")
    out = nc.dram_tensor("out", [P, 16], f32, kind="ExternalOutput")

    cls_rows = []
    r0 = 8
    for nd, *_ in CLS_PLAN:
        cls_rows.append((r0, nd))
        r0 += 2 * nd
    assert r0 == 48

    with ExitStack() as ctx:
        tc = ctx.enter_context(TileContext(nc))
        xb = ctx.enter_context(tc.tile_pool(name="xb", bufs=1))
        work = ctx.enter_context(tc.tile_pool(name="work", bufs=1))
        singles = ctx.enter_context(tc.tile_pool(name="singles", bufs=1))

        acc = singles.tile([P, 16], f32, tag="acc")
        # cols 3 and 9 unused; col 10 is rewritten by the xyP square
        nc.vector.memset(acc[:, 3:4], 0.0)
        nc.vector.memset(acc[:, 9:16], 0.0)

        # ---------------- input tiles ----------------
        xcf = xb.tile([P, 4, C], f16, tag="xcf")      # Tc, Pc
        xwh = xb.tile([P, 2, 4, C], f8, tag="xwh")    # [box][pw,ph,tw,th]
        xxy = xb.tile([P, 2, 4, C], f16, tag="xxy")   # [box][px,py,tx,ty]
        xcls = [
            xb.tile([P, 2 * nd, C], f8, tag=f"xc{i}", name=f"xc{i}")
            for i, (nd, *_) in enumerate(CLS_PLAN)
        ]
        dcls = [
            work.tile([P, nd, C], f16, tag=f"dc{i}", name=f"dc{i}")
            for i, (nd, *_) in enumerate(CLS_PLAN[:4])
        ]
        # tiles 4+5 share one parent so their squares merge into one op
        d45 = work.tile([P, 4, C], f16, tag="d45")
        dcls.append(d45[:, 0:2])
        dcls.append(d45[:, 2:4])

        # ---------------- DMAs ----------------
        # ACT queue: first three (cheap fp8) cls tiles land by ~6us so
        # the sub->mask->square pipeline starts early; SP carries the rest.
        # box1 rows ride the ACT queue so both boxes' chains start by
        # ~6us; cls tiles split across both queues behind them.
        nc.sync.dma_start(out=xcf, in_=xin[:, 0:4])
        nc.sync.dma_start(out=xwh[:, 0], in_=x8[:, 0:4])
        nc.sync.dma_start(out=xxy[:, 0], in_=xin[:, 4:8])
        nc.scalar.dma_start(out=xwh[:, 1], in_=x8[:, 4:8])
        nc.scalar.dma_start(out=xxy[:, 1], in_=xin[:, 8:12])
        for i, (nd, *_n) in enumerate(CLS_PLAN):
            r, _ = cls_rows[i]
            eng = nc.scalar if i in (0, 1) else nc.sync
            eng.dma_start(out=xcls[i], in_=x8[:, r : r + 2 * nd])

        Tc = xcf[:, 0:2]
        Pc = xcf[:, 2:4]
        Pwh = xwh[:, :, 0:2]   # [P, box, (w,h), C]
        Twh = xwh[:, :, 2:4]
        Pxy = xxy[:, :, 0:2]
        Txy = xxy[:, :, 2:4]

        obj = singles.tile([P, C], f16, tag="obj")

        def cls_sub(i, eng):
            nd = CLS_PLAN[i][0]
            xk = xcls[i]
            eng.tensor_tensor(
                out=dcls[i], in0=xk[:, 0:nd], in1=xk[:, nd : 2 * nd],
                op=Op.subtract,
            )

        def cls_mask(i, eng):
            nd = CLS_PLAN[i][0]
            eng.tensor_tensor(
                out=dcls[i], in0=dcls[i],
                in1=obj.unsqueeze(1).broadcast_to([P, nd, C]), op=Op.mult,
            )

        def cls_sq(i):
            col = acc[:, 4 + i : 5 + i]
            nc.scalar.activation(
                out=dcls[i], in_=dcls[i], func=Act.Square, accum_out=col
            )

        # ---------------- DVE conf cluster (emitted first: obj is read
        # by Pool masks) ----------------
        nc.vector.tensor_scalar(
            out=obj, in0=Tc[:, 0], scalar1=0.0, scalar2=None, op0=Op.is_gt
        )
        nobj = singles.tile([P, C], f16, tag="nobj")
        nc.vector.tensor_scalar(
            out=nobj, in0=Tc[:, 0], scalar1=0.0, scalar2=None, op0=Op.is_le
        )
        Dc = work.tile([P, 2, C], f16, tag="Dc")
        nc.vector.tensor_tensor(out=Dc, in0=Pc, in1=Tc, op=Op.subtract)
        gnob = work.tile([P, 2, C], f16, tag="gnob")
        nc.vector.tensor_tensor(
            out=gnob, in0=Dc,
            in1=nobj.unsqueeze(1).broadcast_to([P, 2, C]), op=Op.mult,
        )

        # ---------------- Pool program ----------------
        dwh = work.tile([P, 2, 2, C], f16, tag="dwh")
        swh = work.tile([P, 2, 2, C], f16, tag="swh")
        ar = work.tile([P, 2, 2, C], f16, tag="ar")
        wv = xwh[:, :, 0:4:2]
        hv = xwh[:, :, 1:4:2]

        # wh ops for both boxes first (feed the DVE chain; box1 rows
        # land ~3, box0 ~6), then subs in landing order with the two
        # pool masks placed for ACT's first squares.
        nc.gpsimd.tensor_tensor(
            out=dwh[:, 1], in0=Pwh[:, 1], in1=Twh[:, 1], op=Op.subtract
        )
        nc.gpsimd.tensor_tensor(
            out=swh[:, 1], in0=Pwh[:, 1], in1=Twh[:, 1], op=Op.add
        )
        nc.gpsimd.tensor_tensor(
            out=dwh[:, 0], in0=Pwh[:, 0], in1=Twh[:, 0], op=Op.subtract
        )
        nc.gpsimd.tensor_tensor(
            out=swh[:, 0], in0=Pwh[:, 0], in1=Twh[:, 0], op=Op.add
        )
        nc.gpsimd.tensor_tensor(
            out=ar[:, 1], in0=wv[:, 1], in1=hv[:, 1], op=Op.mult
        )
        nc.gpsimd.tensor_tensor(
            out=ar[:, 0], in0=wv[:, 0], in1=hv[:, 0], op=Op.mult
        )
        cls_sub(0, nc.gpsimd)
        cls_mask(0, nc.gpsimd)
        cls_sub(1, nc.gpsimd)
        cls_mask(1, nc.gpsimd)
        cls_sub(2, nc.gpsimd)
        cls_sub(3, nc.gpsimd)
        cls_sub(4, nc.gpsimd)
        cls_sub(5, nc.gpsimd)

        # ---------------- DVE program ----------------
        # box chain (per box; box0 data lands first)
        dxy = work.tile([P, 2, 2, C], f16, tag="dxy")
        m1 = work.tile([P, 2, 2, C], f16, tag="m1")
        m2 = work.tile([P, 2, 2, C], f16, tag="m2")
        iw = work.tile([P, 2, 2, C], f16, tag="iw")
        r = work.tile([P, 2, 2, C], f16, tag="r")
        inter = work.tile([P, 2, C], f16, tag="inter")

        def chain_box(b):
            nc.vector.tensor_tensor(
                out=dxy[:, b], in0=Pxy[:, b], in1=Txy[:, b], op=Op.subtract
            )
            nc.vector.tensor_tensor(
                out=m1[:, b], in0=dxy[:, b], in1=dwh[:, b], op=Op.max
            )
            nc.vector.tensor_tensor(
                out=m2[:, b], in0=dxy[:, b], in1=dwh[:, b], op=Op.min
            )
            nc.vector.tensor_scalar(
                out=m2[:, b], in0=m2[:, b], scalar1=-1.0, scalar2=None,
                op0=Op.mult,
            )
            nc.vector.tensor_tensor(
                out=m1[:, b], in0=m1[:, b], in1=m2[:, b], op=Op.max
            )
            nc.vector.tensor_tensor(
                out=iw[:, b], in0=swh[:, b], in1=m1[:, b], op=Op.subtract
            )
            nc.vector.tensor_scalar(
                out=r[:, b], in0=iw[:, b], scalar1=0.0, scalar2=None,
                op0=Op.max,
            )
            nc.vector.tensor_tensor(
                out=inter[:, b], in0=r[:, b, 0], in1=r[:, b, 1], op=Op.mult
            )

        chain_box(1)
        chain_box(0)
        s = work.tile([P, 2, C], f16, tag="s")
        nc.vector.tensor_tensor(
            out=s, in0=ar[:, :, 0], in1=ar[:, :, 1], op=Op.add
        )
        lr = work.tile([P, 2, C], f16, tag="lr")
        nc.vector.tensor_tensor(out=lr, in0=inter, in1=s[:, ::-1], op=Op.mult)
        rr = work.tile([P, 2, C], f16, tag="rr")
        nc.vector.tensor_tensor(
            out=rr[:, 0], in0=lr[:, 0], in1=lr[:, 1], op=Op.is_ge
        )
        nc.vector.tensor_scalar(
            out=rr[:, 1], in0=rr[:, 0], scalar1=-1.0, scalar2=1.0,
            op0=Op.mult, op1=Op.add,
        )
        m = work.tile([P, 2, C], f16, tag="m")
        nc.vector.tensor_tensor(
            out=m, in0=rr,
            in1=obj.unsqueeze(1).broadcast_to([P, 2, C]), op=Op.mult,
        )
        mo = m[:, ::-1]

        # masked products interleaved with DVE cls masks in input-
        # readiness order, so ACT's squares stream without FIFO blocks
        gxy = work.tile([P, 2, 2, 2, C], f16, tag="gxy")
        gc = work.tile([P, 2, 2, C], f16, tag="gc")
        nc.vector.tensor_tensor(
            out=gxy[:, 0], in0=dxy,
            in1=m.unsqueeze(2).broadcast_to([P, 2, 2, C]), op=Op.mult,
        )
        cls_mask(2, nc.vector)
        nc.vector.tensor_tensor(
            out=gxy[:, 1], in0=Pxy,
            in1=mo.unsqueeze(2).broadcast_to([P, 2, 2, C]), op=Op.mult,
        )
        cls_mask(3, nc.vector)
        nc.vector.tensor_tensor(out=gc[:, 0], in0=Dc, in1=m, op=Op.mult)
        nc.vector.tensor_tensor(out=gc[:, 1], in0=Pc, in1=mo, op=Op.mult)
        cls_mask(4, nc.vector)
        cls_mask(5, nc.vector)

        # ---------------- ACT program (squares, after its 3 DMAs) ----
        # ordered by expected input readiness so the FIFO never blocks a
        # ready square behind a gated one
        nc.scalar.activation(
            out=gnob, in_=gnob, func=Act.Square, accum_out=acc[:, 2:3]
        )
        cls_sq(0)
        cls_sq(1)
        cls_sq(2)
        nc.scalar.activation(
            out=gxy[:, 0], in_=gxy[:, 0], func=Act.Square,
            accum_out=acc[:, 0:1],
        )
        nc.scalar.activation(
            out=gxy[:, 1], in_=gxy[:, 1], func=Act.Square,
            accum_out=acc[:, 10:11],
        )
        # sq3 as Pool self-mult + DVE accumulate (splits the late square
        # load across engines instead of serializing it all on ACT)
        nc.gpsimd.tensor_tensor(out=dcls[3], in0=dcls[3], in1=dcls[3],
                                op=Op.mult)
        nc.vector.tensor_scalar(
            out=dcls[3].rearrange("p a c -> p (a c)"),
            in0=dcls[3].rearrange("p a c -> p (a c)"),
            scalar1=1.0, scalar2=None,
            op0=Op.mult, op1=Op.add, accum_out=acc[:, 7:8],
        )
        # d45 square as DVE self-mult + tensor_scalar accumulate
        nc.vector.tensor_tensor(out=d45, in0=d45, in1=d45, op=Op.mult)
        nc.vector.tensor_scalar(
            out=d45.rearrange("p a c -> p (a c)"),
            in0=d45.rearrange("p a c -> p (a c)"),
            scalar1=1.0, scalar2=None,
            op0=Op.mult, op1=Op.add, accum_out=acc[:, 8:9],
        )
        # gc square as Pool self-mult + DVE tensor_scalar accumulate
        gc2 = work.tile([P, 2, 2, C], f16, tag="gc2")
        nc.gpsimd.tensor_tensor(out=gc2, in0=gc, in1=gc, op=Op.mult)
        nc.vector.tensor_scalar(
            out=gc2.rearrange("p a b c -> p (a b c)"),
            in0=gc2.rearrange("p a b c -> p (a b c)"),
            scalar1=1.0, scalar2=None,
            op0=Op.mult, op1=Op.add, accum_out=acc[:, 1:2],
        )

        nc.sync.dma_start(out=out[:, :], in_=acc)

    _split_waits(nc)
    return nc


def _get_nc():
    if "nc" not in _CACHE:
        _CACHE["nc"] = _build_bass()
    return _CACHE["nc"]


def _weights():
    w = np.zeros(16, dtype=np.float64)
    w[0] = LAMBDA_COORD / (K_XY * K_XY)
    w[10] = w[0]
    w[1] = 1.0
    w[2] = LAMBDA_NOOBJ       # direct sum of noobj-masked dconf^2
    w[4 : 4 + len(CLS_PLAN)] = 1.0
    return w


def _prep_shards(pred, targ):
    import ml_dtypes

    p = np.asarray(pred, dtype=np.float32).reshape(N_CORES, P, CPP, 30)
    t = np.asarray(targ, dtype=np.float32).reshape(N_CORES, P, CPP, 30)
    X = np.empty((N_CORES, P, 12, CPP), dtype=np.float16)
    X[:, :, 0:2] = t[..., [4, 9]].transpose(0, 1, 3, 2)    # Tc
    X[:, :, 2:4] = p[..., [4, 9]].transpose(0, 1, 3, 2)    # Pc
    X[:, :, 4:6] = p[..., [0, 1]].transpose(0, 1, 3, 2) * K_XY
    X[:, :, 6:8] = t[..., [0, 1]].transpose(0, 1, 3, 2) * K_XY
    X[:, :, 8:10] = p[..., [5, 6]].transpose(0, 1, 3, 2) * K_XY
    X[:, :, 10:12] = t[..., [5, 6]].transpose(0, 1, 3, 2) * K_XY

    X8 = np.empty((N_CORES, P, 48, CPP), dtype=ml_dtypes.float8_e4m3)
    X8[:, :, 0:2] = p[..., [2, 3]].transpose(0, 1, 3, 2)   # pw0, ph0
    X8[:, :, 2:4] = t[..., [2, 3]].transpose(0, 1, 3, 2)   # tw0, th0
    X8[:, :, 4:6] = p[..., [7, 8]].transpose(0, 1, 3, 2)   # pw1, ph1
    X8[:, :, 6:8] = t[..., [7, 8]].transpose(0, 1, 3, 2)   # tw1, th1
    r0, lo = 8, 10
    for nd, *_ in CLS_PLAN:
        X8[:, :, r0 : r0 + nd] = p[..., lo : lo + nd].transpose(0, 1, 3, 2)
        X8[:, :, r0 + nd : r0 + 2 * nd] = t[..., lo : lo + nd].transpose(
            0, 1, 3, 2
        )
        r0 += 2 * nd
        lo += nd
    return [
        (
            np.ascontiguousarray(X[c]),
            np.ascontiguousarray(X8[c]).view(np.uint8),
        )
        for c in range(N_CORES)
    ]


def _host_combine(outs):
    w = _weights()
    total = 0.0
    for o in outs:
        per_f = np.asarray(o, dtype=np.float64).reshape(P, 16).sum(axis=0)
        total += float(per_f @ w)
    return np.float32(total / BATCH)


def _run(inputs, trace=False):
    from concourse.bass_utils import run_bass_kernel_spmd

    shards = _prep_shards(inputs["predictions"], inputs["targets"])
    in_maps = [{"xin": sh[0], "x8": sh[1]} for sh in shards]
    res = run_bass_kernel_spmd(
        _get_nc(), in_maps, core_ids=list(range(N_CORES)), trace=trace
    )
    loss = _host_combine([r["out"] for r in res.results])
    return loss, res


def kernel(predictions, targets):
    loss, _ = _run({"predictions": predictions, "targets": targets})
    return loss
